# revision 3
# baseline (speedup 1.0000x reference)
"""BitNet attention (B=2, S=2048, HID=2560, NH=20, NKV=5, HD=128, GQA=4) on 8 TRN2 cores.

Sharding: 2-way batch x 4-way head-group tensor parallel.
Core (b, g) computes q-heads [4g, 4g+1, 4g+2, 4g+3, 16+g] and kv-heads [g, 4]
for batch b (uniform q-head -> kv mapping across cores so one SPMD NEFF works:
local heads 0-3 -> kv slot 0, local head 4 -> kv slot 1).

Per-core device pipeline (everything transposed, feature-on-partitions):
  P1: Q^T = Wq@X^T, K^T = Wk@X^T (bf16 matmuls, exact ternary weights), RoPE on DVE
  P2: V = X@Wv^T in natural (seq, hd) layout
  P3: per (head, seq-block): S^T = K^T.T@Q^T (fp32r), exp on ACT (scale=alpha),
      causal binary mask on diag tiles, AV + softmax-denominator (ones-matmul)
      accumulated in PSUM, normalize via K=1 broadcast matmul + reciprocal,
      per-position sum-of-squares (for RMSNorm) via ones-matmul,
      o-proj partials y^T = Wo'@(w * attn_out^T) in bf16.
Host: unpack ternary weights, build RoPE tables, all-reduce partial y / sumsq,
      apply softmax-free scales + RMSNorm scale (per-seq scalars commute through
      the linear o-proj).
"""

import math
import numpy as np
import ml_dtypes
from contextlib import ExitStack

import concourse.bacc as bacc
import concourse.tile as tile
import concourse.mybir as mybir
from concourse import bass_utils

# problem constants (hardcoded per contract)
B, S, HID = 2, 2048, 2560
NH, NKV, HD = 20, 5, 128
THETA = 500000.0
RMS_EPS = 1e-6

N_CORES = 8
KT = HID // 128          # 20 k-tiles over hidden dim
J = S // 512             # 4 seq blocks of 512
SKT = S // 128           # 16 sk tiles
NQH = 5                  # q heads per core
NKVH = 2                 # kv heads per core

F32 = mybir.dt.float32
F32R = mybir.dt.float32r
BF16 = mybir.dt.bfloat16
F16 = mybir.dt.float16

_cache = {}


def _build(alpha: float, repeats: int):
    nc = bacc.Bacc("TRN2", target_bir_lowering=False, debug=False, num_devices=N_CORES)

    # DRAM I/O
    xt_d = nc.dram_tensor("xt", [HID, S], BF16, kind="ExternalInput")
    wq_d = nc.dram_tensor("wq", [HID, NQH * HD], BF16, kind="ExternalInput")
    wk_d = nc.dram_tensor("wk", [HID, NKVH * HD], BF16, kind="ExternalInput")
    wv_d = nc.dram_tensor("wv", [HID, NKVH * HD], BF16, kind="ExternalInput")
    wo_d = nc.dram_tensor("wo", [NQH * HD, HID], BF16, kind="ExternalInput")
    cos_d = nc.dram_tensor("cos", [HD, S], F16, kind="ExternalInput")
    sin_d = nc.dram_tensor("sin", [HD, S], F16, kind="ExternalInput")
    wn_d = nc.dram_tensor("wn", [HD, NQH], F32, kind="ExternalInput")
    dmask_d = nc.dram_tensor("dmask", [4, HD, 512], F32R, kind="ExternalInput")
    onc_d = nc.dram_tensor("onc", [HD, 1], F32R, kind="ExternalInput")
    onr_d = nc.dram_tensor("onr", [1, HD], F32R, kind="ExternalInput")
    y_d = nc.dram_tensor("y", [HID, S], F32, kind="ExternalOutput")
    ssq_d = nc.dram_tensor("ssq", [1, S], F32, kind="ExternalOutput")

    with tile.TileContext(nc) as tc, ExitStack() as octx:
        # persistent pools (live across the whole iteration)
        ps = octx.enter_context(tc.tile_pool(name="ps", bufs=8, space="PSUM"))
        qt_p = octx.enter_context(tc.tile_pool(name="qt", bufs=1))
        kt_p = octx.enter_context(tc.tile_pool(name="kt", bufs=1))
        v_p = octx.enter_context(tc.tile_pool(name="v", bufs=1))
        const_p = octx.enter_context(tc.tile_pool(name="const", bufs=1))

        def body(_it=None):
            with ExitStack() as ctx:
                # constants
                onc = const_p.tile([HD, 1], F32R, tag="onc")
                nc.sync.dma_start(onc[:], onc_d.ap())
                onr = const_p.tile([1, HD], F32R, tag="onr")
                nc.sync.dma_start(onr[:], onr_d.ap())
                wn = const_p.tile([HD, NQH], F32, tag="wn")
                nc.sync.dma_start(wn[:], wn_d.ap())
                dmask = const_p.tile([HD, 4 * 512], F32R, tag="dmask")
                for o in range(4):
                    nc.sync.dma_start(dmask[:, o * 512:(o + 1) * 512], dmask_d.ap()[o])

                qt = qt_p.tile([128, NQH * S], F32R, tag="qt")      # Q^T, 5 heads
                kt = kt_p.tile([128, NKVH * S], F32R, tag="kt")     # K^T, 2 kv heads
                vt = v_p.tile([128, SKT * NKVH * HD], F32R, tag="vt")  # V natural, 16 sk tiles

                # ---------------- P1 + P2: projections ----------------
                with ExitStack() as pctx:
                    w_p = pctx.enter_context(tc.tile_pool(name="wqkv", bufs=1))
                    xt_p = pctx.enter_context(tc.tile_pool(name="xt", bufs=2))
                    tab_p = pctx.enter_context(tc.tile_pool(name="tab", bufs=1))
                    rp_p = pctx.enter_context(tc.tile_pool(name="rp", bufs=4))

                    cos_t = tab_p.tile([HD, S], F16, tag="cos")
                    nc.sync.dma_start(cos_t[:], cos_d.ap())
                    sin_t = tab_p.tile([HD, S], F16, tag="sin")
                    nc.sync.dma_start(sin_t[:], sin_d.ap())

                    # weights, one big strided DMA each: [p, k*W + o] = w[k*128+p, o]
                    wq = w_p.tile([128, KT * NQH * HD], BF16, tag="wq")
                    nc.sync.dma_start(
                        wq[:].rearrange("p (k o) -> p k o", k=KT),
                        wq_d.ap().rearrange("(k p) o -> p k o", p=128))
                    wk = w_p.tile([128, KT * NKVH * HD], BF16, tag="wk")
                    nc.sync.dma_start(
                        wk[:].rearrange("p (k o) -> p k o", k=KT),
                        wk_d.ap().rearrange("(k p) o -> p k o", p=128))
                    wv = w_p.tile([128, KT * NKVH * HD], BF16, tag="wv")
                    nc.sync.dma_start(
                        wv[:].rearrange("p (k o) -> p k o", k=KT),
                        wv_d.ap().rearrange("(k p) o -> p k o", p=128))

                    for j in range(J):
                        sq = slice(j * 512, (j + 1) * 512)
                        xt = xt_p.tile([128, KT * 512], BF16, tag="xt")
                        nc.sync.dma_start(
                            xt[:].rearrange("p (k s) -> p k s", k=KT),
                            xt_d.ap()[:, sq].rearrange("(k p) s -> p k s", p=128))

                        ps_q = [ps.tile([128, 512], F32, tag="ps", name=f"psq{m}") for m in range(NQH)]
                        ps_k = [ps.tile([128, 512], F32, tag="ps", name=f"psk{m}") for m in range(NKVH)]
                        for k in range(KT):
                            xk = xt[:, k * 512:(k + 1) * 512]
                            st, sp = (k == 0), (k == KT - 1)
                            for m in range(NQH):
                                nc.tensor.matmul(
                                    ps_q[m][:],
                                    wq[:, k * 640 + m * 128: k * 640 + (m + 1) * 128],
                                    xk, start=st, stop=sp)
                            for m in range(NKVH):
                                nc.tensor.matmul(
                                    ps_k[m][:],
                                    wk[:, k * 256 + m * 128: k * 256 + (m + 1) * 128],
                                    xk, start=st, stop=sp)

                        # RoPE: out = x*cos + rot(x)*sin_signed
                        for m, (psrc, dst) in enumerate(
                                [(ps_q[i], qt[:, i * S + j * 512: i * S + (j + 1) * 512])
                                 for i in range(NQH)] +
                                [(ps_k[i], kt[:, i * S + j * 512: i * S + (j + 1) * 512])
                                 for i in range(NKVH)]):
                            tcos = rp_p.tile([128, 512], F32, tag="tcos")
                            nc.vector.tensor_mul(tcos[:], psrc[:], cos_t[:, sq])
                            trot = rp_p.tile([128, 512], F32, tag="trot")
                            nc.vector.tensor_mul(
                                trot[0:64, :], psrc[64:128, :], sin_t[0:64, sq])
                            nc.vector.tensor_mul(
                                trot[64:128, :], psrc[0:64, :], sin_t[64:128, sq])
                            nc.vector.tensor_add(dst, tcos[:], trot[:])

                        # P2: V for sk tiles of this block (natural layout)
                        ps_v = [ps.tile([128, NKVH * HD], F32, tag="ps", name=f"psv{t}") for t in range(4)]
                        for k in range(KT):
                            st, sp = (k == 0), (k == KT - 1)
                            for t in range(4):
                                nc.tensor.matmul(
                                    ps_v[t][:],
                                    xt[:, k * 512 + t * 128: k * 512 + (t + 1) * 128],
                                    wv[:, k * 256:(k + 1) * 256],
                                    start=st, stop=sp)
                        for t in range(4):
                            i = 4 * j + t
                            nc.scalar.copy(
                                vt[:, i * 256:(i + 1) * 256], ps_v[t][:])

                # ---------------- P3: attention + o-proj ----------------
                with ExitStack() as actx:
                    wo_p = actx.enter_context(tc.tile_pool(name="wo", bufs=1))
                    pr_p = actx.enter_context(tc.tile_pool(name="probs", bufs=6))
                    tw_p = actx.enter_context(tc.tile_pool(name="tw", bufs=7))
                    mis_p = actx.enter_context(tc.tile_pool(name="mis", bufs=3))
                    y_p = actx.enter_context(tc.tile_pool(name="ysb", bufs=3))

                    wo = wo_p.tile([128, NQH * HID], BF16, tag="wo")
                    nc.sync.dma_start(
                        wo[:].rearrange("p (h o) -> p h o", h=NQH),
                        wo_d.ap().rearrange("(h p) o -> p h o", p=128))

                    for j in range(J):
                        ni = 4 * j + 4  # active sk tiles
                        ssq_ps = ps.tile([1, 512], F32, tag="ps")
                        tws = []
                        for h in range(NQH):
                            kvl = 0 if h < 4 else 1
                            qr = qt[:, h * S + j * 512: h * S + (j + 1) * 512]
                            av_ps = ps.tile([128, 512], F32, tag="ps")
                            d_ps = ps.tile([1, 512], F32, tag="ps")
                            prev = None
                            for i in range(ni):
                                s_ps = ps.tile([128, 512], F32, tag="ps")
                                nc.tensor.matmul(
                                    s_ps[:],
                                    kt[:, kvl * S + i * 128: kvl * S + (i + 1) * 128],
                                    qr, start=True, stop=True)
                                if prev is not None:  # 1-deep SW pipeline on PE
                                    pp, pi = prev
                                    nc.tensor.matmul(
                                        av_ps[:],
                                        vt[:, pi * 256 + kvl * 128: pi * 256 + kvl * 128 + 128],
                                        pp[:], start=(pi == 0), stop=(pi == ni - 1))
                                    nc.tensor.matmul(
                                        d_ps[:], onc[:], pp[:],
                                        start=(pi == 0), stop=(pi == ni - 1))
                                probs = pr_p.tile([128, 512], F32R, tag="probs")
                                nc.scalar.activation(
                                    probs[:], s_ps[:],
                                    mybir.ActivationFunctionType.Exp, scale=alpha)
                                if i >= 4 * j:
                                    o = i - 4 * j
                                    nc.vector.tensor_mul(
                                        probs[:], probs[:],
                                        dmask[:, o * 512:(o + 1) * 512])
                                prev = (probs, i)
                            pp, pi = prev
                            nc.tensor.matmul(
                                av_ps[:],
                                vt[:, pi * 256 + kvl * 128: pi * 256 + kvl * 128 + 128],
                                pp[:], start=(pi == 0), stop=(pi == ni - 1))
                            nc.tensor.matmul(
                                d_ps[:], onc[:], pp[:],
                                start=(pi == 0), stop=(pi == ni - 1))

                            # normalize + w + sumsq
                            drow = mis_p.tile([1, 512], F32R, tag="drow")
                            nc.scalar.copy(drow[:], d_ps[:])
                            bc_ps = ps.tile([128, 512], F32, tag="ps")
                            nc.tensor.matmul(
                                bc_ps[:], onr[:], drow[:], start=True, stop=True)
                            rec = mis_p.tile([128, 512], F32, tag="rec")
                            nc.vector.reciprocal(rec[:], bc_ps[:])
                            tn = mis_p.tile([128, 512], F32, tag="tn")
                            nc.vector.tensor_mul(tn[:], av_ps[:], rec[:])
                            sqt = mis_p.tile([128, 512], F32R, tag="sqt")
                            nc.scalar.square(sqt[:], tn[:])
                            nc.tensor.matmul(
                                ssq_ps[:], onc[:], sqt[:],
                                start=(h == 0), stop=(h == NQH - 1))
                            tw = tw_p.tile([128, 512], BF16, tag="tw")
                            nc.vector.tensor_scalar_mul(tw[:], tn[:], wn[:, h:h + 1])
                            tws.append(tw)

                        srow = mis_p.tile([1, 512], F32, tag="srow")
                        nc.scalar.copy(srow[:], ssq_ps[:])
                        nc.sync.dma_start(ssq_d.ap()[:, j * 512:(j + 1) * 512], srow[:])

                        # o-proj for this block
                        for m in range(KT):
                            y_ps = ps.tile([128, 512], F32, tag="ps")
                            for h in range(NQH):
                                nc.tensor.matmul(
                                    y_ps[:],
                                    wo[:, h * HID + m * 128: h * HID + (m + 1) * 128],
                                    tws[h][:], start=(h == 0), stop=(h == NQH - 1))
                            ysb = y_p.tile([128, 512], F32, tag="ysb")
                            nc.scalar.copy(ysb[:], y_ps[:])
                            nc.sync.dma_start(
                                y_d.ap()[m * 128:(m + 1) * 128, j * 512:(j + 1) * 512],
                                ysb[:])

        if repeats > 1:
            with tc.For_i(0, repeats) as _i:
                body(_i)
        else:
            body()

    nc.compile()
    return nc


def _unpack_ternary(packed: np.ndarray) -> np.ndarray:
    M, Kp = packed.shape
    nb = Kp // 32
    b = packed.reshape(M, nb, 32)
    f = np.stack([(b >> 6) & 3, (b >> 4) & 3, (b >> 2) & 3, b & 3], axis=2)
    return f.reshape(M, nb * 128).astype(np.float32) - 1.0


def _rope_tables():
    inv = 1.0 / (THETA ** (np.arange(0, HD, 2, dtype=np.float64) / HD))  # (64,)
    t = np.arange(S, dtype=np.float64)
    fr = t[None, :] * inv[:, None]          # (64, S)
    cos = np.concatenate([np.cos(fr), np.cos(fr)], axis=0)      # (128, S)
    sin = np.concatenate([-np.sin(fr), np.sin(fr)], axis=0)     # signed
    return cos.astype(np.float16), sin.astype(np.float16)


def _diag_masks():
    m = np.zeros((4, HD, 512), dtype=np.float32)
    q = np.arange(512)[None, :]
    p = np.arange(HD)[:, None]
    for o in range(4):
        m[o] = (q >= p + 128 * o).astype(np.float32)
    return m


def make_in_maps(hidden_states, q_w, k_w, v_w, o_w, attn_norm_w):
    wq_f = _unpack_ternary(np.asarray(q_w))     # (2560, 2560)
    wk_f = _unpack_ternary(np.asarray(k_w))     # (640, 2560)
    wv_f = _unpack_ternary(np.asarray(v_w))     # (640, 2560)
    wo_f = _unpack_ternary(np.asarray(o_w))     # (2560, 2560) [out, in]
    cos, sin = _rope_tables()
    dmask = _diag_masks()
    onc = np.ones((HD, 1), np.float32)
    onr = np.ones((1, HD), np.float32)
    wnorm = np.asarray(attn_norm_w, np.float32)
    hs = np.asarray(hidden_states)

    bf = ml_dtypes.bfloat16
    in_maps = []
    for c in range(N_CORES):
        b, g = c // 4, c % 4
        qheads = [4 * g, 4 * g + 1, 4 * g + 2, 4 * g + 3, 16 + g]
        kvheads = [g, 4]
        qrows = np.concatenate([wq_f[h * HD:(h + 1) * HD] for h in qheads], 0)
        krows = np.concatenate([wk_f[h * HD:(h + 1) * HD] for h in kvheads], 0)
        vrows = np.concatenate([wv_f[h * HD:(h + 1) * HD] for h in kvheads], 0)
        ocols = np.concatenate([wo_f[:, h * HD:(h + 1) * HD] for h in qheads], 1)
        wn = np.stack([wnorm[h * HD:(h + 1) * HD] for h in qheads], 1)  # (128, 5)
        in_maps.append({
            "xt": np.ascontiguousarray(hs[b].T).astype(bf),
            "wq": np.ascontiguousarray(qrows.T).astype(bf),
            "wk": np.ascontiguousarray(krows.T).astype(bf),
            "wv": np.ascontiguousarray(vrows.T).astype(bf),
            "wo": np.ascontiguousarray(ocols.T).astype(bf),
            "cos": cos, "sin": sin,
            "wn": np.ascontiguousarray(wn),
            "dmask": dmask, "onc": onc, "onr": onr,
        })
    return in_maps


def postprocess(results, v_scale, o_scale):
    out = np.empty((B, S, HID), np.float32)
    for b in range(B):
        y = np.zeros((HID, S), np.float64)
        ss = np.zeros((S,), np.float64)
        for g in range(4):
            r = results[b * 4 + g]
            y += r["y"].astype(np.float64)
            ss += r["ssq"][0].astype(np.float64)
        var = ss * (float(v_scale) ** 2) / HID
        rms = 1.0 / np.sqrt(var + RMS_EPS)
        out[b] = (y.T * (rms[:, None] * float(v_scale) * float(o_scale))).astype(np.float32)
    return out


def _get_nc(alpha: float, repeats: int = 1):
    key = (round(alpha, 12), repeats)
    if key not in _cache:
        _cache[key] = _build(alpha, repeats)
    return _cache[key]


def kernel(hidden_states, attention_mask, q_w, k_w, v_w, o_w,
           q_scale, k_scale, v_scale, o_scale, attn_norm_w):
    alpha = float(q_scale) * float(k_scale) / math.sqrt(HD)
    nc = _get_nc(alpha, 1)
    in_maps = make_in_maps(hidden_states, q_w, k_w, v_w, o_w, attn_norm_w)
    res = bass_utils.run_bass_kernel_spmd(nc, in_maps, core_ids=list(range(N_CORES)))
    return postprocess(res.results, v_scale, o_scale)


# revision 5
# speedup vs baseline: 2.0224x; 2.0224x over previous
"""BitNet attention (B=2, S=2048, HID=2560, NH=20, NKV=5, HD=128, GQA=4) on 8 TRN2 cores.

Sharding: 2-way batch x 4-way head-group tensor parallel.
Core (b, g) computes q-heads [4g, 4g+1, 4g+2, 4g+3, 16+g] and kv-heads [g, 4]
for batch b (uniform q-head -> kv mapping across cores so one SPMD NEFF works:
local heads 0-3 -> kv slot 0, local head 4 -> kv slot 1).

Per-core device pipeline, fused per 512-wide seq block j (causal => attention
for block j only needs K/V of blocks <= j):
  - Q^T/K^T = W@X^T (bf16 matmuls, exact ternary weights), RoPE on DVE
  - V = X@Wv^T in natural (seq, hd) layout
  - per head: S^T = K^T.T@Q^T (fp32r), exp on ACT (scale=alpha), causal binary
    mask on diag tiles, AV + softmax-denominator (ones-matmul) in PSUM,
    normalize via K=1 broadcast matmul + reciprocal, sum-of-squares for RMSNorm
    via ones-matmul; per-head tail chains are software-pipelined one head late.
  - o-proj partials y^T = Wo'@(w * attn_out^T) in bf16.
Host: unpack ternary weights, build RoPE tables, sum partial y / sumsq over the
4 cores of each batch, apply v/o scales and the RMSNorm per-seq scale (per-seq
scalars commute through the linear o-proj).
"""

import math
import numpy as np
import ml_dtypes
from contextlib import ExitStack

import concourse.bacc as bacc
import concourse.tile as tile
import concourse.mybir as mybir
from concourse import bass_utils

B, S, HID = 2, 2048, 2560
NH, NKV, HD = 20, 5, 128
THETA = 500000.0
RMS_EPS = 1e-6

N_CORES = 8
KT = HID // 128          # 20 k-tiles over hidden dim
J = S // 512             # 4 seq blocks of 512
SKT = S // 128           # 16 sk tiles
NQH = 5                  # q heads per core
NKVH = 2                 # kv heads per core

F32 = mybir.dt.float32
F32R = mybir.dt.float32r
BF16 = mybir.dt.bfloat16
F16 = mybir.dt.float16

_cache = {}


def _build(alpha: float, repeats: int):
    nc = bacc.Bacc("TRN2", target_bir_lowering=False, debug=False, num_devices=N_CORES)

    xt_d = nc.dram_tensor("xt", [HID, S], BF16, kind="ExternalInput")
    wq_d = nc.dram_tensor("wq", [HID, NQH * HD], BF16, kind="ExternalInput")
    wk_d = nc.dram_tensor("wk", [HID, NKVH * HD], BF16, kind="ExternalInput")
    wv_d = nc.dram_tensor("wv", [HID, NKVH * HD], BF16, kind="ExternalInput")
    wo_d = nc.dram_tensor("wo", [NQH * HD, HID], BF16, kind="ExternalInput")
    cos_d = nc.dram_tensor("cos", [HD, S], F16, kind="ExternalInput")
    sin_d = nc.dram_tensor("sin", [HD, S], F16, kind="ExternalInput")
    wn_d = nc.dram_tensor("wn", [HD, NQH], F32, kind="ExternalInput")
    dmask_d = nc.dram_tensor("dmask", [4, HD, 512], F32R, kind="ExternalInput")
    onc_d = nc.dram_tensor("onc", [HD, 1], F32R, kind="ExternalInput")
    onr_d = nc.dram_tensor("onr", [1, HD], F32R, kind="ExternalInput")
    y_d = nc.dram_tensor("y", [HID, S], F32, kind="ExternalOutput")
    ssq_d = nc.dram_tensor("ssq", [1, S], F32, kind="ExternalOutput")

    with tile.TileContext(nc) as tc, ExitStack() as octx:
        ps = octx.enter_context(tc.tile_pool(name="ps", bufs=8, space="PSUM"))
        kt_p = octx.enter_context(tc.tile_pool(name="ktp", bufs=1))
        v_p = octx.enter_context(tc.tile_pool(name="vp", bufs=1))
        qb_p = octx.enter_context(tc.tile_pool(name="qbp", bufs=6))
        const_p = octx.enter_context(tc.tile_pool(name="constp", bufs=1))
        w_p = octx.enter_context(tc.tile_pool(name="wp", bufs=1))
        xt_p = octx.enter_context(tc.tile_pool(name="xtp", bufs=1))
        rp_p = octx.enter_context(tc.tile_pool(name="rpp", bufs=2))
        pr_p = octx.enter_context(tc.tile_pool(name="prp", bufs=4))
        tw_p = octx.enter_context(tc.tile_pool(name="twp", bufs=7))
        mis_p = octx.enter_context(tc.tile_pool(name="misp", bufs=2))
        y_p = octx.enter_context(tc.tile_pool(name="yp", bufs=2))

        def body(_it=None):
            # --- persistent SBUF for one iteration ---
            kt = kt_p.tile([128, NKVH * S], F32R, tag="kt", name="kt")
            vt = v_p.tile([128, SKT * NKVH * HD], F32R, tag="vt", name="vt")

            wq = w_p.tile([128, KT * NQH * HD], BF16, tag="wq", name="wq")
            wk = w_p.tile([128, KT * NKVH * HD], BF16, tag="wk", name="wk")
            wv = w_p.tile([128, KT * NKVH * HD], BF16, tag="wv", name="wv")
            wo = w_p.tile([128, NQH * HID], BF16, tag="wo", name="wo")

            def dma_w_chunk(dst, src_d, W, k0, k1):
                nc.sync.dma_start(
                    dst[:, k0 * W:k1 * W].rearrange("p (k o) -> p k o", k=k1 - k0),
                    src_d.ap()[k0 * 128:k1 * 128].rearrange("(k p) o -> p k o", p=128))

            def dma_xt_chunk(dst, j, k0, k1):
                nc.sync.dma_start(
                    dst[:, k0 * 512:k1 * 512].rearrange("p (k s) -> p k s", k=k1 - k0),
                    xt_d.ap()[k0 * 128:k1 * 128, j * 512:(j + 1) * 512]
                    .rearrange("(k p) s -> p k s", p=128))

            xts = [None] * J

            # first compute chunk's data first, then the rest interleaved
            xts[0] = xt_p.tile([128, KT * 512], BF16, tag="xt", name="xt0")
            dma_xt_chunk(xts[0], 0, 0, 5)
            dma_w_chunk(wq, wq_d, NQH * HD, 0, 5)
            dma_w_chunk(wk, wk_d, NKVH * HD, 0, 5)
            dma_w_chunk(wv, wv_d, NKVH * HD, 0, 5)
            for c in range(1, 4):
                dma_xt_chunk(xts[0], 0, 5 * c, 5 * c + 5)
                dma_w_chunk(wq, wq_d, NQH * HD, 5 * c, 5 * c + 5)
                dma_w_chunk(wk, wk_d, NKVH * HD, 5 * c, 5 * c + 5)
                dma_w_chunk(wv, wv_d, NKVH * HD, 5 * c, 5 * c + 5)

            # constants / tables (needed slightly later than the first matmuls)
            cos_t = const_p.tile([HD, S], F16, tag="cos", name="cos")
            nc.sync.dma_start(cos_t[:], cos_d.ap())
            sin_t = const_p.tile([HD, S], F16, tag="sin", name="sin")
            nc.sync.dma_start(sin_t[:], sin_d.ap())
            onc = const_p.tile([HD, 1], F32R, tag="onc", name="onc")
            nc.sync.dma_start(onc[:], onc_d.ap())
            onr = const_p.tile([1, HD], F32R, tag="onr", name="onr")
            nc.sync.dma_start(onr[:], onr_d.ap())
            wn = const_p.tile([HD, NQH], F32, tag="wn", name="wn")
            nc.sync.dma_start(wn[:], wn_d.ap())
            dmask = const_p.tile([HD, 4 * 512], F32R, tag="dmask", name="dmask")
            for o in range(4):
                nc.sync.dma_start(dmask[:, o * 512:(o + 1) * 512], dmask_d.ap()[o])

            pending = [None]  # (h, j, av_ps, d_ps, ssq_ps, tws)

            def emit_tail():
                if pending[0] is None:
                    return
                h, j, av_ps, d_ps, ssq_ps, tws = pending[0]
                pending[0] = None
                drow = mis_p.tile([1, 512], F32R, tag="drow", name=f"dr{j}_{h}")
                nc.scalar.copy(drow[:], d_ps[:])
                bc_ps = ps.tile([128, 512], F32, tag="ps", name=f"bc{j}_{h}")
                nc.tensor.matmul(bc_ps[:], onr[:], drow[:], start=True, stop=True)
                rec = mis_p.tile([128, 512], F32, tag="rec", name=f"rc{j}_{h}")
                nc.vector.reciprocal(rec[:], bc_ps[:])
                tn = mis_p.tile([128, 512], F32, tag="tn", name=f"tn{j}_{h}")
                nc.vector.tensor_mul(tn[:], av_ps[:], rec[:])
                sqt = mis_p.tile([128, 512], F32R, tag="sqt", name=f"sq{j}_{h}")
                nc.scalar.square(sqt[:], tn[:])
                nc.tensor.matmul(ssq_ps[:], onc[:], sqt[:],
                                 start=(h == 0), stop=(h == NQH - 1))
                tw = tw_p.tile([128, 512], BF16, tag="tw", name=f"tw{j}_{h}")
                nc.vector.tensor_scalar_mul(tw[:], tn[:], wn[:, h:h + 1])
                tws.append(tw)

            for j in range(J):
                sq = slice(j * 512, (j + 1) * 512)
                xt = xts[j]
                if xt is None:
                    xt = xts[j] = xt_p.tile([128, KT * 512], BF16, tag="xt",
                                            name=f"xt{j}")
                    for c in range(4):
                        dma_xt_chunk(xt, j, 5 * c, 5 * c + 5)

                # ---- projections q/k for this block ----
                qbs = [qb_p.tile([128, 512], F32R, tag="qb", name=f"qb{j}_{h}")
                       for h in range(NQH)]
                ps_q = [ps.tile([128, 512], F32, tag="ps", name=f"pq{j}_{m}")
                        for m in range(NQH)]
                ps_k = [ps.tile([128, 512], F32, tag="ps", name=f"pk{j}_{m}")
                        for m in range(NKVH)]
                for k in range(KT):
                    xk = xt[:, k * 512:(k + 1) * 512]
                    st, sp = (k == 0), (k == KT - 1)
                    for m in range(NQH):
                        nc.tensor.matmul(
                            ps_q[m][:],
                            wq[:, k * 640 + m * 128: k * 640 + (m + 1) * 128],
                            xk, start=st, stop=sp)
                    for m in range(NKVH):
                        nc.tensor.matmul(
                            ps_k[m][:],
                            wk[:, k * 256 + m * 128: k * 256 + (m + 1) * 128],
                            xk, start=st, stop=sp)
                if j == 0:
                    # wo needed only at the first o-proj; start its DMA now
                    nc.sync.dma_start(
                        wo[:].rearrange("p (h o) -> p h o", h=NQH),
                        wo_d.ap().rearrange("(h p) o -> p h o", p=128))

                # ---- RoPE ----
                targets = ([(ps_q[i], qbs[i][:]) for i in range(NQH)]
                           + [(ps_k[i], kt[:, i * S + j * 512: i * S + (j + 1) * 512])
                              for i in range(NKVH)])
                for idx, (psrc, dst) in enumerate(targets):
                    trot = rp_p.tile([128, 512], F32, tag="trot", name=f"tr{j}_{idx}")
                    nc.vector.tensor_mul(trot[0:64, :], psrc[64:128, :], sin_t[0:64, sq])
                    nc.vector.tensor_mul(trot[64:128, :], psrc[0:64, :], sin_t[64:128, sq])
                    nc.vector.tensor_mul(dst, psrc[:], cos_t[:, sq])
                    nc.vector.tensor_add(dst, dst, trot[:])

                # ---- V for this block ----
                ps_v = [ps.tile([128, NKVH * HD], F32, tag="ps", name=f"pv{j}_{t}")
                        for t in range(4)]
                for k in range(KT):
                    st, sp = (k == 0), (k == KT - 1)
                    for t in range(4):
                        nc.tensor.matmul(
                            ps_v[t][:],
                            xt[:, k * 512 + t * 128: k * 512 + (t + 1) * 128],
                            wv[:, k * 256:(k + 1) * 256],
                            start=st, stop=sp)
                for t in range(4):
                    i = 4 * j + t
                    nc.scalar.copy(vt[:, i * 256:(i + 1) * 256], ps_v[t][:])

                # ---- attention ----
                ni = 4 * j + 4
                ssq_ps = ps.tile([1, 512], F32, tag="ps", name=f"pss{j}")
                tws = []
                for h in range(NQH):
                    kvl = 0 if h < 4 else 1
                    qr = qbs[h][:]
                    av_ps = ps.tile([128, 512], F32, tag="ps", name=f"pav{j}_{h}")
                    d_ps = ps.tile([1, 512], F32, tag="ps", name=f"pd{j}_{h}")
                    prev = None
                    for i in range(ni):
                        s_ps = ps.tile([128, 512], F32, tag="ps", name=f"pS{j}_{h}_{i}")
                        nc.tensor.matmul(
                            s_ps[:],
                            kt[:, kvl * S + i * 128: kvl * S + (i + 1) * 128],
                            qr, start=True, stop=True)
                        if prev is not None:
                            pp, pi = prev
                            nc.tensor.matmul(
                                av_ps[:],
                                vt[:, pi * 256 + kvl * 128: pi * 256 + kvl * 128 + 128],
                                pp[:], start=(pi == 0), stop=(pi == ni - 1))
                            nc.tensor.matmul(
                                d_ps[:], onc[:], pp[:],
                                start=(pi == 0), stop=(pi == ni - 1))
                        probs = pr_p.tile([128, 512], F32R, tag="probs",
                                          name=f"pr{j}_{h}_{i}")
                        nc.scalar.activation(
                            probs[:], s_ps[:],
                            mybir.ActivationFunctionType.Exp, scale=alpha)
                        if i >= 4 * j:
                            o = i - 4 * j
                            nc.vector.tensor_mul(
                                probs[:], probs[:], dmask[:, o * 512:(o + 1) * 512])
                        if i == 0:
                            emit_tail()  # previous head's tail, overlapped
                        prev = (probs, i)
                    pp, pi = prev
                    nc.tensor.matmul(
                        av_ps[:],
                        vt[:, pi * 256 + kvl * 128: pi * 256 + kvl * 128 + 128],
                        pp[:], start=(pi == 0), stop=(pi == ni - 1))
                    nc.tensor.matmul(
                        d_ps[:], onc[:], pp[:],
                        start=(pi == 0), stop=(pi == ni - 1))
                    pending[0] = (h, j, av_ps, d_ps, ssq_ps, tws)
                emit_tail()  # last head of the block

                srow = mis_p.tile([1, 512], F32, tag="srow", name=f"sr{j}")
                nc.scalar.copy(srow[:], ssq_ps[:])
                nc.sync.dma_start(ssq_d.ap()[:, sq], srow[:])

                # ---- o-proj, m-chunked ----
                for mc in range(0, KT, 4):
                    y_pss = [ps.tile([128, 512], F32, tag="ps", name=f"py{j}_{m}")
                             for m in range(mc, mc + 4)]
                    for h in range(NQH):
                        for mi, m in enumerate(range(mc, mc + 4)):
                            nc.tensor.matmul(
                                y_pss[mi][:],
                                wo[:, h * HID + m * 128: h * HID + (m + 1) * 128],
                                tws[h][:], start=(h == 0), stop=(h == NQH - 1))
                    for mi, m in enumerate(range(mc, mc + 4)):
                        ysb = y_p.tile([128, 512], F32, tag="ysb", name=f"y{j}_{m}")
                        nc.scalar.copy(ysb[:], y_pss[mi][:])
                        nc.sync.dma_start(
                            y_d.ap()[m * 128:(m + 1) * 128, sq], ysb[:])

                # prefetch next block's activations
                if j + 1 < J:
                    xts[j + 1] = xt_p.tile([128, KT * 512], BF16, tag="xt",
                                           name=f"xt{j+1}")
                    for c in range(4):
                        dma_xt_chunk(xts[j + 1], j + 1, 5 * c, 5 * c + 5)

        if repeats > 1:
            with tc.For_i(0, repeats) as _i:
                body(_i)
        else:
            body()

    nc.compile()
    return nc


def _unpack_ternary(packed: np.ndarray) -> np.ndarray:
    M, Kp = packed.shape
    nb = Kp // 32
    b = packed.reshape(M, nb, 32)
    f = np.stack([(b >> 6) & 3, (b >> 4) & 3, (b >> 2) & 3, b & 3], axis=2)
    return f.reshape(M, nb * 128).astype(np.float32) - 1.0


def _rope_tables():
    inv = 1.0 / (THETA ** (np.arange(0, HD, 2, dtype=np.float64) / HD))  # (64,)
    t = np.arange(S, dtype=np.float64)
    fr = t[None, :] * inv[:, None]          # (64, S)
    cos = np.concatenate([np.cos(fr), np.cos(fr)], axis=0)      # (128, S)
    sin = np.concatenate([-np.sin(fr), np.sin(fr)], axis=0)     # signed
    return cos.astype(np.float16), sin.astype(np.float16)


def _diag_masks():
    m = np.zeros((4, HD, 512), dtype=np.float32)
    q = np.arange(512)[None, :]
    p = np.arange(HD)[:, None]
    for o in range(4):
        m[o] = (q >= p + 128 * o).astype(np.float32)
    return m


def make_in_maps(hidden_states, q_w, k_w, v_w, o_w, attn_norm_w):
    wq_f = _unpack_ternary(np.asarray(q_w))     # (2560, 2560)
    wk_f = _unpack_ternary(np.asarray(k_w))     # (640, 2560)
    wv_f = _unpack_ternary(np.asarray(v_w))     # (640, 2560)
    wo_f = _unpack_ternary(np.asarray(o_w))     # (2560, 2560) [out, in]
    cos, sin = _rope_tables()
    dmask = _diag_masks()
    onc = np.ones((HD, 1), np.float32)
    onr = np.ones((1, HD), np.float32)
    wnorm = np.asarray(attn_norm_w, np.float32)
    hs = np.asarray(hidden_states)

    bf = ml_dtypes.bfloat16
    in_maps = []
    for c in range(N_CORES):
        b, g = c // 4, c % 4
        qheads = [4 * g, 4 * g + 1, 4 * g + 2, 4 * g + 3, 16 + g]
        kvheads = [g, 4]
        qrows = np.concatenate([wq_f[h * HD:(h + 1) * HD] for h in qheads], 0)
        krows = np.concatenate([wk_f[h * HD:(h + 1) * HD] for h in kvheads], 0)
        vrows = np.concatenate([wv_f[h * HD:(h + 1) * HD] for h in kvheads], 0)
        ocols = np.concatenate([wo_f[:, h * HD:(h + 1) * HD] for h in qheads], 1)
        wn = np.stack([wnorm[h * HD:(h + 1) * HD] for h in qheads], 1)  # (128, 5)
        in_maps.append({
            "xt": np.ascontiguousarray(hs[b].T).astype(bf),
            "wq": np.ascontiguousarray(qrows.T).astype(bf),
            "wk": np.ascontiguousarray(krows.T).astype(bf),
            "wv": np.ascontiguousarray(vrows.T).astype(bf),
            "wo": np.ascontiguousarray(ocols.T).astype(bf),
            "cos": cos, "sin": sin,
            "wn": np.ascontiguousarray(wn),
            "dmask": dmask, "onc": onc, "onr": onr,
        })
    return in_maps


def postprocess(results, v_scale, o_scale):
    out = np.empty((B, S, HID), np.float32)
    for b in range(B):
        y = np.zeros((HID, S), np.float64)
        ss = np.zeros((S,), np.float64)
        for g in range(4):
            r = results[b * 4 + g]
            y += r["y"].astype(np.float64)
            ss += r["ssq"][0].astype(np.float64)
        var = ss * (float(v_scale) ** 2) / HID
        rms = 1.0 / np.sqrt(var + RMS_EPS)
        out[b] = (y.T * (rms[:, None] * float(v_scale) * float(o_scale))).astype(np.float32)
    return out


def _get_nc(alpha: float, repeats: int = 1):
    key = (round(alpha, 12), repeats)
    if key not in _cache:
        _cache[key] = _build(alpha, repeats)
    return _cache[key]


def kernel(hidden_states, attention_mask, q_w, k_w, v_w, o_w,
           q_scale, k_scale, v_scale, o_scale, attn_norm_w):
    alpha = float(q_scale) * float(k_scale) / math.sqrt(HD)
    nc = _get_nc(alpha, 1)
    in_maps = make_in_maps(hidden_states, q_w, k_w, v_w, o_w, attn_norm_w)
    res = bass_utils.run_bass_kernel_spmd(nc, in_maps, core_ids=list(range(N_CORES)))
    return postprocess(res.results, v_scale, o_scale)
